# revision 2
# baseline (speedup 1.0000x reference)
"""Trainium2 Bass kernel for nn_CombinedCS (FISTA compressed-sensing recon).

Self-contained: hardcodes shapes (B=16, H=W=320), shards batch over 8 cores
(2 images per core), runs the full 15-iteration FISTA loop SBUF-resident.

Math plan (validated vs reference in numpy):
  - centered 2D FFT as two PE matmul stages against the DFT matrix F
    (transpose-free: data is always lhsT, F^T always rhs)
  - TV prox (5 Chambolle iters): h-direction div/grad as PE left-mults by
    bidiagonal matrices; w-direction via shifted free-dim views with zero
    guard columns; inner loop in bf16 with q = lam*p scaling
  - 3-level Haar DWT: w-step unnormalized (a+b, a-b) on DVE, h-step as PE
    left-mult by orthonormal Haar matrix; detail soft-threshold at
    lam*sqrt(2)^level; inverse folds the w-step 1/2 into the h-step matrix

Layout P6: one complex image (2 ch x 320 x 320) packs into
[128 partitions, 6 blocks, 320]; channel ch occupies blocks 3ch..3ch+2
with h = 128*qb + p (qb block-in-channel; block 3ch+2 uses p<64 only).
All matmul outputs are partition-base-0 (ISA requirement).
"""
import math
import os

import ml_dtypes
import numpy as np

H = W = 320
B = 16
NCORES = 8
IMGS = B // NCORES  # 2
LAM_TV = 0.005
LAM_WAV = 0.005
TAU = 0.25
TV_ITERS = 5
LEVELS = 3
MAX_ITER = int(os.environ.get("CS_ITERS", "15"))
SKIP_TV = os.environ.get("CS_SKIP_TV", "0") == "1"
SKIP_DWT = os.environ.get("CS_SKIP_DWT", "0") == "1"
S2 = math.sqrt(2.0)

# layouts: per ch, list of (p0, p1, q, r0, r1): matrix rows r0..r1 live at
# partitions p0..p1 of block q. All tiles base-0 (matmul dst requirement).
P6D = {
    0: [(0, 128, 0, 0, 128), (0, 128, 1, 128, 256), (0, 64, 2, 256, 320)],
    1: [(0, 128, 3, 0, 128), (0, 128, 4, 128, 256), (0, 64, 5, 256, 320)],
}
P6C = [(0, 128, 0, 0, 128), (0, 128, 1, 128, 256), (0, 64, 2, 256, 320)]
L2D = {
    0: [(0, 128, 0, 0, 128), (0, 32, 1, 128, 160)],
    1: [(0, 128, 2, 0, 128), (0, 32, 3, 128, 160)],
}
L2C = [(0, 128, 0, 0, 128), (0, 32, 1, 128, 160)]
L3D = {
    0: [(0, 64, 0, 0, 64), (0, 16, 1, 64, 80)],
    1: [(0, 64, 2, 0, 64), (0, 16, 3, 64, 80)],
}
L3C = [(0, 64, 0, 0, 64), (0, 16, 1, 64, 80)]


def _dft_mats():
    I = np.eye(H, dtype=np.complex128)
    F = np.fft.fftshift(
        np.fft.fft(np.fft.ifftshift(I, axes=0), axis=0, norm="ortho"), axes=0
    )
    G = np.conj(F).T
    return F, G


def _tv_mats():
    Dd = np.zeros((H, H))
    Dd[0, 0] = 1.0
    for h in range(1, H - 1):
        Dd[h, h] = 1.0
        Dd[h, h - 1] = -1.0
    Dd[H - 1, H - 2] = -1.0
    Dg = np.zeros((H, H))
    for h in range(H - 1):
        Dg[h, h] = -1.0
        Dg[h, h + 1] = 1.0
    return Dd, Dg


def _haar_mat(n):
    Wm = np.zeros((n, n))
    hn = n // 2
    c = 1.0 / S2
    for i in range(hn):
        Wm[i, 2 * i] = c
        Wm[i, 2 * i + 1] = c
        Wm[hn + i, 2 * i] = c
        Wm[hn + i, 2 * i + 1] = -c
    return Wm


def _momentum_coeffs():
    t = 1.0
    out = []
    for _ in range(MAX_ITER):
        t_new = (1.0 + math.sqrt(1.0 + 4.0 * t * t)) / 2.0
        out.append((t - 1.0) / t_new)
        t = t_new
    return out


def _pack_p6(x):
    """x: (2, 320, 320) -> (128, 6, 320), zero-padded dead region."""
    out = np.zeros((128, 6, 320), dtype=x.dtype)
    for ch in range(2):
        out[:, 3 * ch + 0] = x[ch, 0:128]
        out[:, 3 * ch + 1] = x[ch, 128:256]
        out[0:64, 3 * ch + 2] = x[ch, 256:320]
    return out


def _unpack_p6(p):
    out = np.zeros((2, 320, 320), dtype=p.dtype)
    for ch in range(2):
        out[ch, 0:128] = p[:, 3 * ch + 0]
        out[ch, 128:256] = p[:, 3 * ch + 1]
        out[ch, 256:320] = p[0:64, 3 * ch + 2]
    return out


def _host_consts():
    F, G = _dft_mats()
    Dd, Dg = _tv_mats()
    W1, W2, W3 = _haar_mat(320), _haar_mat(160), _haar_mat(80)
    f32 = np.float32
    bf16 = ml_dtypes.bfloat16
    return {
        "ftr": F.real.T.astype(f32), "fti": F.imag.T.astype(f32),
        "ftin": (-F.imag.T).astype(f32),
        "ifr": G.real.T.astype(f32), "ifi": G.imag.T.astype(f32),
        "ifin": (-G.imag.T).astype(f32),
        "ddt": Dd.T.astype(f32), "dgt": Dg.T.astype(f32),
        "w1t": W1.T.astype(f32), "w1h": (0.5 * W1).astype(f32),
        "w2t": W2.T.astype(f32), "w2h": (0.5 * W2).astype(f32),
        "w3t": W3.T.astype(f32), "w3h": (0.5 * W3).astype(f32),
    }


def _copy_segs(src_lay, dst_lay, nrows):
    out = {}
    for ch in (0, 1):
        def locate(lay, r):
            for (p0, p1, q, r0, r1) in lay[ch]:
                if r0 <= r < r1:
                    return p0 + (r - r0), q, r1 - r
            raise AssertionError(r)
        segs = []
        r = 0
        while r < nrows:
            sp, sq, sleft = locate(src_lay, r)
            dp, dq, dleft = locate(dst_lay, r)
            cnt = min(sleft, dleft, nrows - r)
            segs.append((sp, sq, dp, dq, cnt))
            r += cnt
        out[ch] = segs
    return out


SEG12 = _copy_segs(P6D, L2D, 160)
SEG23 = _copy_segs(L2D, L3D, 80)


def _build_nc():
    import concourse.bacc as bacc
    import concourse.tile as tile
    import concourse.mybir as mybir
    from contextlib import ExitStack

    dt = mybir.dt
    F32, F32R, BF16 = dt.float32, dt.float32r, dt.bfloat16
    ALU = mybir.AluOpType
    AF = mybir.ActivationFunctionType

    s_tv = TAU * LAM_TV
    lam = LAM_TV
    eps_q = lam * lam * 1e-8
    lam1 = lam / s_tv
    eps1 = eps_q / (s_tv * s_tv)
    coeffs = _momentum_coeffs()
    lam_lvl = [LAM_WAV * (S2 ** (l + 1)) for l in range(LEVELS)]

    nc = bacc.Bacc("TRN2", target_bir_lowering=False, debug=False,
                   num_devices=NCORES)

    dr = {}
    for name in ("ftr", "fti", "ftin", "ifr", "ifi", "ifin", "w1t", "w1h"):
        dr[name] = nc.dram_tensor(name, [320, 320], F32R, kind="ExternalInput").ap()
    for name in ("w2t", "w2h"):
        dr[name] = nc.dram_tensor(name, [160, 160], F32R, kind="ExternalInput").ap()
    for name in ("w3t", "w3h"):
        dr[name] = nc.dram_tensor(name, [80, 80], F32R, kind="ExternalInput").ap()
    for name in ("ddt", "dgt"):
        dr[name] = nc.dram_tensor(name, [320, 320], F32R, kind="ExternalInput").ap()
    dr["zz"] = nc.dram_tensor("zz", [128, 6, 320], F32R, kind="ExternalInput").ap()
    for i in range(IMGS):
        dr[f"y{i}"] = nc.dram_tensor(f"y{i}", [128, 6, 320], F32R, kind="ExternalInput").ap()
        dr[f"ym{i}"] = nc.dram_tensor(f"ym{i}", [128, 6, 320], F32, kind="ExternalInput").ap()
        dr[f"mk{i}"] = nc.dram_tensor(f"mk{i}", [128, 6, 320], BF16, kind="ExternalInput").ap()
        dr[f"xo{i}"] = nc.dram_tensor(f"xo{i}", [128, 6, 320], F32, kind="ExternalOutput").ap()

    with ExitStack() as ctx:
        tc = ctx.enter_context(tile.TileContext(nc))
        st = ctx.enter_context(tc.tile_pool(name="state", bufs=1))
        psa = ctx.enter_context(tc.tile_pool(name="psa", bufs=1, space="PSUM"))
        psb = ctx.enter_context(tc.tile_pool(name="psb", bufs=1, space="PSUM"))

        def T(tag, shape, dtp):
            return st.tile(shape, dtp, tag=tag, name=tag)

        cv = {}
        for name in ("ftr", "fti", "ftin", "ifr", "ifi", "ifin", "w1t", "w1h"):
            cv[name] = T("c_" + name, [128, 3, 320], F32R)
        for name in ("ddt", "dgt"):
            cv[name] = T("c_" + name, [128, 3, 320], F32R)
        for name in ("w2t", "w2h"):
            cv[name] = T("c_" + name, [128, 2, 160], F32R)
        for name in ("w3t", "w3h"):
            cv[name] = T("c_" + name, [128, 2, 80], F32R)

        def load_const(name, lay):
            for (p0, p1, q, r0, r1) in lay:
                nc.sync.dma_start(cv[name][p0:p1, q, :], dr[name][r0:r1, :])

        for name in ("ftr", "fti", "ftin", "ifr", "ifi", "ifin", "w1t", "w1h",
                     "ddt", "dgt"):
            load_const(name, P6C)
        for name in ("w2t", "w2h"):
            load_const(name, L2C)
        for name in ("w3t", "w3h"):
            load_const(name, L3C)

        per_img = []
        for i in range(IMGS):
            per_img.append({
                "z": T(f"z{i}", [128, 6, 320], F32R),
                "xA": T(f"xA{i}", [128, 6, 320], F32),
                "xB": T(f"xB{i}", [128, 6, 320], F32),
                "ym": T(f"ymk{i}", [128, 6, 320], F32),
                "mk": T(f"msk{i}", [128, 6, 320], BF16),
            })
        sbA = T("sbA", [128, 6, 320], F32R)
        Km = T("Km", [128, 6, 320], F32R)
        xcb = T("xcb", [128, 6, 320], F32R)
        wtmp = T("wtmp", [128, 6, 320], F32R)
        Y1 = T("Y1", [128, 6, 320], F32R)
        Y2 = T("Y2", [128, 4, 160], F32R)
        Y3 = T("Y3", [128, 4, 80], F32R)
        L2t = T("L2t", [128, 4, 160], F32R)
        L3t = T("L3t", [128, 4, 80], F32R)
        qx = T("qx", [128, 6, 322], F32)
        qy = T("qy", [128, 6, 320], F32R)
        vt = T("vt", [128, 6, 320], F32R)
        tv1 = T("tv1", [128, 6, 320], F32)
        n2 = T("n2", [128, 6, 320], F32)
        rr = T("rr", [128, 6, 320], F32)
        sgn = T("sgn", [128, 6, 320], BF16)

        def psA():
            return psa.tile([128, 6, 512], F32, tag="A", name="psA")

        def psB():
            return psb.tile([128, 4, 256], F32, tag="B", name="psB")

        nc.vector.memset(qx[:], 0.0)
        nc.sync.dma_start(qy[:], dr["zz"][:])

        # ----- pair views: valid region of a P6-layout tensor as 2 APs -----
        def pv(t, c0=0, c1=None, wd=None):
            c1 = c1 if c1 is not None else (wd if wd is not None else t.shape[-1])
            r = t.rearrange("p (g b) w -> p g b w", g=2)
            return [r[0:128, 0, 0:2, c0:c1], r[0:128, 1, 0:2, c0:c1],
                    r[0:64, :, 2, c0:c1]]

        def pvs(t, c0, c1, step):
            """strided-column pair views (for DWT w ops)"""
            r = t.rearrange("p (g b) w -> p g b w", g=2)
            return [r[0:128, 0, 0:2, c0:c1:step], r[0:128, 1, 0:2, c0:c1:step],
                    r[0:64, :, 2, c0:c1:step]]

        def ew(fn, *views):
            for i in range(3):
                fn(*[v[i] for v in views])

        STT = nc.vector.scalar_tensor_tensor
        TT = nc.vector.tensor_tensor

        def soft_views(views, lam_l, m_views, sgn_views):
            for i in range(len(views)):
                nc.scalar.activation(m_views[i], views[i], AF.Abs)
                nc.scalar.activation(sgn_views[i], views[i], AF.Sign)
                nc.vector.tensor_scalar(m_views[i], m_views[i], lam_l, lam_l,
                                        ALU.max, ALU.subtract)
                nc.vector.tensor_tensor(views[i], m_views[i], sgn_views[i], ALU.mult)

        # ---------- matmul emitters ----------
        def mm_fft(ps, data, terms, ncols=320):
            for oc in (0, 1):
                for (mp0, mp1, mq, mr0, mr1) in P6D[oc]:
                    mml = []
                    for (dch, cname) in terms[oc]:
                        cvt = cv[cname]
                        for t in range(3):
                            dp0, dp1, dq, _, _ = P6D[dch][t]
                            cp0, cp1, cq, _, _ = P6C[t]
                            mml.append((data[dp0:dp1, dq, mr0:mr1],
                                        cvt[cp0:cp1, cq, 0:ncols]))
                    n = len(mml)
                    for idx, (l, r) in enumerate(mml):
                        nc.tensor.matmul(ps[mp0:mp1, mq, 0:ncols], l, r,
                                         start=(idx == 0), stop=(idx == n - 1))

        def mm_left(ps, cname, data, dlay, clay, ncols):
            for ch in (0, 1):
                cvt = cv[cname]
                dts = dlay[ch]
                n = len(dts)
                for (mp0, mp1, mq, mr0, mr1) in dlay[ch]:
                    for t in range(n):
                        dp0, dp1, dq, _, _ = dts[t]
                        cp0, cp1, cq, _, _ = clay[t]
                        nc.tensor.matmul(
                            ps[mp0:mp1, mq, 0:ncols],
                            cvt[cp0:cp1, cq, mr0:mr1],
                            data[dp0:dp1, dq, 0:ncols],
                            start=(t == 0), stop=(t == n - 1))

        def fft2(data, consts, out_ps):
            cr, ci, cin = consts
            p1 = psA()
            mm_fft(p1, data, {0: [(0, cr), (1, cin)], 1: [(0, ci), (1, cr)]})
            ew(nc.scalar.copy, pv(sbA), pv(p1, c1=320))
            mm_fft(out_ps, sbA, {0: [(0, cr), (1, cin)], 1: [(0, ci), (1, cr)]})

        FWD = ("ftr", "fti", "ftin")
        INV = ("ifr", "ifi", "ifin")

        # ---------- init ----------
        for i in range(IMGS):
            im = per_img[i]
            nc.sync.dma_start(Km[:], dr[f"y{i}"][:])
            nc.sync.dma_start(im["ym"][:], dr[f"ym{i}"][:])
            nc.sync.dma_start(im["mk"][:], dr[f"mk{i}"][:])
            pG = psA()
            fft2(Km, INV, pG)
            ew(nc.scalar.copy, pv(im["xA"]), pv(pG, c1=320))
            ew(nc.vector.tensor_copy, pv(im["z"]), pv(pG, c1=320))

        # ---------- FISTA ----------
        for k in range(MAX_ITER):
            for i in range(IMGS):
                im = per_img[i]
                xold = im["xA"] if k % 2 == 0 else im["xB"]
                xnew = im["xB"] if k % 2 == 0 else im["xA"]

                pK = psA()
                fft2(im["z"], FWD, pK)
                ew(lambda o, a, b: TT(o, a, b, ALU.mult),
                   pv(Km), pv(pK, c1=320), pv(im["mk"]))
                ew(lambda o, a, b: TT(o, a, b, ALU.subtract),
                   pv(Km), pv(Km), pv(im["ym"]))
                pG = psA()
                fft2(Km, INV, pG)
                xc = Km
                ew(lambda o, a, b: TT(o, a, b, ALU.subtract),
                   pv(xc), pv(im["z"]), pv(pG, c1=320))
                ew(nc.scalar.copy, pv(xcb), pv(xc))

                # ---------- TV prox ----------
                for it in range(0 if SKIP_TV else TV_ITERS):
                    if it == 0:
                        pT = psA()
                        mm_left(pT, "dgt", xcb, P6D, P6C, 320)
                        ew(lambda o, a, b: TT(o, a, b, ALU.subtract),
                           pv(qx, 2, 321), pv(xcb, 1, 320), pv(xcb, 0, 319))
                        ew(nc.scalar.square, pv(tv1), pv(qx, 2, 322))
                        ew(nc.scalar.square, pv(vt), pv(pT, c1=320))
                        ew(lambda o, a, b: STT(o, a, eps1, b, ALU.add, ALU.add),
                           pv(n2), pv(tv1), pv(vt))
                        ew(lambda o, a: nc.vector.tensor_scalar_max(o, a, lam1 * lam1),
                           pv(n2), pv(n2))
                        ew(lambda o, a: nc.scalar.activation(
                            o, a, AF.Abs_reciprocal_sqrt, scale=1.0 / (lam * lam)),
                           pv(rr), pv(n2))
                        ew(lambda o, a, b: TT(o, a, b, ALU.mult),
                           pv(qx, 2, 321), pv(qx, 2, 321), pv(rr, 0, 319))
                        ew(lambda o, a, b: TT(o, a, b, ALU.mult),
                           pv(qy), pv(pT, c1=320), pv(rr))
                    else:
                        ew(lambda o, a, b: STT(o, a, -1.0, b, ALU.mult, ALU.add),
                           pv(tv1), pv(qx, 2, 322), pv(xcb))
                        ew(lambda o, a, b: TT(o, a, b, ALU.add),
                           pv(tv1), pv(tv1), pv(qx, 1, 321))
                        pT = psA()
                        mm_left(pT, "ddt", qy, P6D, P6C, 320)
                        ew(lambda o, a, b: TT(o, a, b, ALU.subtract),
                           pv(vt), pv(tv1), pv(pT, c1=320))
                        pT2 = psA()
                        mm_left(pT2, "dgt", vt, P6D, P6C, 320)
                        ew(lambda o, a, b: STT(o, a, s_tv, b, ALU.mult, ALU.add),
                           pv(qx, 2, 321), pv(vt, 1, 320), pv(qx, 2, 321))
                        ew(lambda o, a, b: STT(o, a, -s_tv, b, ALU.mult, ALU.add),
                           pv(qx, 2, 321), pv(vt, 0, 319), pv(qx, 2, 321))
                        ew(lambda o, a, b: STT(o, a, s_tv, b, ALU.mult, ALU.add),
                           pv(qy), pv(pT2, c1=320), pv(qy))
                        ew(nc.scalar.square, pv(tv1), pv(qx, 2, 322))
                        ew(nc.scalar.square, pv(vt), pv(qy))
                        ew(lambda o, a, b: STT(o, a, eps_q, b, ALU.add, ALU.add),
                           pv(n2), pv(tv1), pv(vt))
                        ew(lambda o, a: nc.vector.tensor_scalar_max(o, a, lam * lam),
                           pv(n2), pv(n2))
                        ew(lambda o, a: nc.scalar.activation(
                            o, a, AF.Abs_reciprocal_sqrt, scale=1.0 / (lam * lam)),
                           pv(rr), pv(n2))
                        ew(lambda o, a, b: TT(o, a, b, ALU.mult),
                           pv(qx, 2, 321), pv(qx, 2, 321), pv(rr, 0, 319))
                        ew(lambda o, a, b: TT(o, a, b, ALU.mult),
                           pv(qy), pv(qy), pv(rr))
                xtv = sbA
                if SKIP_TV:
                    ew(nc.vector.tensor_copy, pv(xtv), pv(xc))
                else:
                    pT = psA()
                    mm_left(pT, "ddt", qy, P6D, P6C, 320)
                    ew(lambda o, a, b: STT(o, a, -1.0, b, ALU.mult, ALU.add),
                       pv(xtv), pv(qx, 2, 322), pv(xc))
                    ew(lambda o, a, b: TT(o, a, b, ALU.add),
                       pv(xtv), pv(xtv), pv(qx, 1, 321))
                    ew(lambda o, a, b: TT(o, a, b, ALU.subtract),
                       pv(xtv), pv(xtv), pv(pT, c1=320))

                if SKIP_DWT:
                    ew(nc.vector.tensor_copy, pv(xnew), pv(xtv))
                else:
                    # ---------- DWT forward ----------
                    ew(lambda o, a, b: TT(o, a, b, ALU.add),
                       pv(wtmp, 0, 160), pvs(xtv, 0, 320, 2), pvs(xtv, 1, 320, 2))
                    ew(lambda o, a, b: TT(o, a, b, ALU.subtract),
                       pv(wtmp, 160, 320), pvs(xtv, 0, 320, 2), pvs(xtv, 1, 320, 2))
                    pY = psA()
                    mm_left(pY, "w1t", wtmp, P6D, P6C, 320)
                    ew(nc.scalar.copy, pv(Y1), pv(pY, c1=320))
                    for ch in (0, 1):
                        for (sp, sq, dp, dq, cnt) in SEG12[ch]:
                            TT(L2t[dp:dp + cnt, dq, 0:80],
                               Y1[sp:sp + cnt, sq, 0:160:2],
                               Y1[sp:sp + cnt, sq, 1:160:2], ALU.add)
                            TT(L2t[dp:dp + cnt, dq, 80:160],
                               Y1[sp:sp + cnt, sq, 0:160:2],
                               Y1[sp:sp + cnt, sq, 1:160:2], ALU.subtract)
                    pY2 = psB()
                    mm_left(pY2, "w2t", L2t, L2D, L2C, 160)
                    nc.scalar.copy(Y2[0:128, 0:4:2, :], pY2[0:128, 0:4:2, 0:160])
                    nc.scalar.copy(Y2[0:32, 1:4:2, :], pY2[0:32, 1:4:2, 0:160])
                    for ch in (0, 1):
                        for (sp, sq, dp, dq, cnt) in SEG23[ch]:
                            TT(L3t[dp:dp + cnt, dq, 0:40],
                               Y2[sp:sp + cnt, sq, 0:80:2],
                               Y2[sp:sp + cnt, sq, 1:80:2], ALU.add)
                            TT(L3t[dp:dp + cnt, dq, 40:80],
                               Y2[sp:sp + cnt, sq, 0:80:2],
                               Y2[sp:sp + cnt, sq, 1:80:2], ALU.subtract)
                    pY3 = psB()
                    mm_left(pY3, "w3t", L3t, L3D, L3C, 80)
                    nc.scalar.copy(Y3[0:64, 0:4:2, :], pY3[0:64, 0:4:2, 0:80])
                    nc.scalar.copy(Y3[0:16, 1:4:2, :], pY3[0:16, 1:4:2, 0:80])
                    # thresholds: save ll3, soft-threshold everything, restore ll3
                    nc.scalar.copy(L3t[0:40, 0:4:2, 0:40], Y3[0:40, 0:4:2, 0:40])
                    soft_views([Y3[0:64, 0:4:2, :], Y3[0:16, 1:4:2, :]], lam_lvl[2],
                               [wtmp[0:64, 0:4:2, 0:80], wtmp[0:16, 1:4:2, 0:80]],
                               [sgn[0:64, 0:4:2, 0:80], sgn[0:16, 1:4:2, 0:80]])
                    nc.scalar.copy(Y3[0:40, 0:4:2, 0:40], L3t[0:40, 0:4:2, 0:40])
                    soft_views([Y2[0:128, 0:4:2, :], Y2[0:32, 1:4:2, :]], lam_lvl[1],
                               [wtmp[0:128, 0:4:2, 0:160], wtmp[0:32, 1:4:2, 0:160]],
                               [sgn[0:128, 0:4:2, 0:160], sgn[0:32, 1:4:2, 0:160]])
                    soft_views(pv(Y1), lam_lvl[0], pv(wtmp), pv(sgn))

                    # ---------- DWT inverse ----------
                    pZ3 = psB()
                    mm_left(pZ3, "w3h", Y3, L3D, L3C, 80)
                    nc.scalar.copy(L3t[0:64, 0:4:2, :], pZ3[0:64, 0:4:2, 0:80])
                    nc.scalar.copy(L3t[0:16, 1:4:2, :], pZ3[0:16, 1:4:2, 0:80])
                    for ch in (0, 1):
                        for (sp, sq, dp, dq, cnt) in SEG23[ch]:
                            TT(Y2[sp:sp + cnt, sq, 0:80:2],
                               L3t[dp:dp + cnt, dq, 0:40],
                               L3t[dp:dp + cnt, dq, 40:80], ALU.add)
                            TT(Y2[sp:sp + cnt, sq, 1:80:2],
                               L3t[dp:dp + cnt, dq, 0:40],
                               L3t[dp:dp + cnt, dq, 40:80], ALU.subtract)
                    pZ2 = psB()
                    mm_left(pZ2, "w2h", Y2, L2D, L2C, 160)
                    nc.scalar.copy(L2t[0:128, 0:4:2, :], pZ2[0:128, 0:4:2, 0:160])
                    nc.scalar.copy(L2t[0:32, 1:4:2, :], pZ2[0:32, 1:4:2, 0:160])
                    for ch in (0, 1):
                        for (sp, sq, dp, dq, cnt) in SEG12[ch]:
                            TT(Y1[sp:sp + cnt, sq, 0:160:2],
                               L2t[dp:dp + cnt, dq, 0:80],
                               L2t[dp:dp + cnt, dq, 80:160], ALU.add)
                            TT(Y1[sp:sp + cnt, sq, 1:160:2],
                               L2t[dp:dp + cnt, dq, 0:80],
                               L2t[dp:dp + cnt, dq, 80:160], ALU.subtract)
                    pZ1 = psA()
                    mm_left(pZ1, "w1h", Y1, P6D, P6C, 320)
                    ew(nc.scalar.copy, pv(wtmp), pv(pZ1, c1=320))
                    ew(lambda o, a, b: TT(o, a, b, ALU.add),
                       pvs(xnew, 0, 320, 2), pv(wtmp, 0, 160), pv(wtmp, 160, 320))
                    ew(lambda o, a, b: TT(o, a, b, ALU.subtract),
                       pvs(xnew, 1, 320, 2), pv(wtmp, 0, 160), pv(wtmp, 160, 320))

                # ---------- momentum ----------
                if k < MAX_ITER - 1:
                    ew(lambda o, a, b: TT(o, a, b, ALU.subtract),
                       pv(wtmp), pv(xnew), pv(xold))
                    ew(lambda o, a, b: STT(o, a, coeffs[k], b, ALU.mult, ALU.add),
                       pv(im["z"]), pv(wtmp), pv(xnew))

        fin = "xB" if (MAX_ITER - 1) % 2 == 0 else "xA"
        for i in range(IMGS):
            nc.sync.dma_start(dr[f"xo{i}"][:], per_img[i][fin][:])

    nc.compile()
    return nc


_NC = None


def _get_nc():
    global _NC
    if _NC is None:
        _NC = _build_nc()
    return _NC


def _build_in_maps(y, mask):
    c = _host_consts()
    in_maps = []
    for core in range(NCORES):
        m = dict(c)
        m["zz"] = np.zeros((128, 6, 320), dtype=np.float32)
        for i in range(IMGS):
            b = core * IMGS + i
            mpair = np.broadcast_to(mask[b], (2, 320, 320)).astype(np.float32)
            m[f"y{i}"] = _pack_p6(y[b])
            m[f"ym{i}"] = _pack_p6((mask[b] * y[b]).astype(np.float32))
            m[f"mk{i}"] = _pack_p6(mpair).astype(ml_dtypes.bfloat16)
        in_maps.append(m)
    return in_maps


def kernel(y, mask):
    from concourse.bass_utils import run_bass_kernel_spmd

    y = np.asarray(y, dtype=np.float32)
    mask = np.asarray(mask, dtype=np.float32)
    nc = _get_nc()
    in_maps = _build_in_maps(y, mask)

    res = run_bass_kernel_spmd(nc, in_maps, list(range(NCORES)))
    out = np.zeros((B, 2, H, W), dtype=np.float32)
    for core in range(NCORES):
        for i in range(IMGS):
            out[core * IMGS + i] = _unpack_p6(res.results[core][f"xo{i}"])
    return out



# revision 5
# speedup vs baseline: 1.2955x; 1.2955x over previous
"""Trainium2 Bass kernel for nn_CombinedCS (FISTA compressed-sensing recon).

Self-contained: hardcodes shapes (B=16, H=W=320), shards batch over 8 cores
(2 images per core), runs the full 15-iteration FISTA loop SBUF-resident.

Math plan (validated vs reference in numpy):
  - centered 2D FFT as two PE matmul stages against the DFT matrix F
    (transpose-free: data is always lhsT, F^T always rhs)
  - TV prox (5 Chambolle iters): h-direction div/grad as PE left-mults by
    bidiagonal matrices (zero blocks skipped); w-direction via shifted
    free-dim views with zero guard columns; dual p tracked unscaled
    (p-units) so fp16 stays in normal range
  - 3-level Haar DWT: w-step unnormalized (a+b, a-b) on DVE, h-step as PE
    left-mult by orthonormal Haar matrix; detail soft-threshold at
    lam*sqrt(2)^level via x - clamp(x, +-t); inverse folds the w-step 1/2
    into the h-step matrix

All matmul operands are float16 (1 cycle/row on PE vs 4 for fp32 HIGH);
x/z iterates and the final TV/momentum arithmetic stay fp32.

Layout P6: one complex image (2 ch x 320 x 320) packs into
[128 partitions, 6 blocks, 320]; channel ch occupies blocks 3ch..3ch+2
with h = 128*qb + p (qb block-in-channel; block 3ch+2 uses p<64 only).
All matmul outputs are partition-base-0 (ISA requirement).
"""
import math
import os

import numpy as np

H = W = 320
B = 16
NCORES = 8
IMGS = B // NCORES  # 2
LAM_TV = 0.005
LAM_WAV = 0.005
TAU = 0.25
TV_ITERS = 5
LEVELS = 3
MAX_ITER = int(os.environ.get("CS_ITERS", "15"))
SKIP_TV = os.environ.get("CS_SKIP_TV", "0") == "1"
SKIP_DWT = os.environ.get("CS_SKIP_DWT", "0") == "1"
S2 = math.sqrt(2.0)

# layouts: per ch, list of (p0, p1, q, r0, r1): matrix rows r0..r1 live at
# partitions p0..p1 of block q. All tiles base-0 (matmul dst requirement).
P6D = {
    0: [(0, 128, 0, 0, 128), (0, 128, 1, 128, 256), (0, 64, 2, 256, 320)],
    1: [(0, 128, 3, 0, 128), (0, 128, 4, 128, 256), (0, 64, 5, 256, 320)],
}
P6C = [(0, 128, 0, 0, 128), (0, 128, 1, 128, 256), (0, 64, 2, 256, 320)]
L2D = {
    0: [(0, 128, 0, 0, 128), (0, 32, 1, 128, 160)],
    1: [(0, 128, 2, 0, 128), (0, 32, 3, 128, 160)],
}
L2C = [(0, 128, 0, 0, 128), (0, 32, 1, 128, 160)]
L3D = {
    0: [(0, 64, 0, 0, 64), (0, 16, 1, 64, 80)],
    1: [(0, 64, 2, 0, 64), (0, 16, 3, 64, 80)],
}
L3C = [(0, 64, 0, 0, 64), (0, 16, 1, 64, 80)]

# non-zero (out_tile, contract_tile) block pairs of the banded TV matrices
# (Dd lower-bidiagonal, Dg upper-bidiagonal) and the Haar W1 matrix
TV_PAIRS = {(0, 0), (1, 0), (1, 1), (2, 1), (2, 2)}
W1_PAIRS = {(0, 0), (0, 1), (1, 0), (1, 1), (1, 2), (2, 1), (2, 2)}


def _dft_mats():
    I = np.eye(H, dtype=np.complex128)
    F = np.fft.fftshift(
        np.fft.fft(np.fft.ifftshift(I, axes=0), axis=0, norm="ortho"), axes=0
    )
    G = np.conj(F).T
    return F, G


def _tv_mats():
    Dd = np.zeros((H, H))
    Dd[0, 0] = 1.0
    for h in range(1, H - 1):
        Dd[h, h] = 1.0
        Dd[h, h - 1] = -1.0
    Dd[H - 1, H - 2] = -1.0
    Dg = np.zeros((H, H))
    for h in range(H - 1):
        Dg[h, h] = -1.0
        Dg[h, h + 1] = 1.0
    return Dd, Dg


def _haar_mat(n):
    Wm = np.zeros((n, n))
    hn = n // 2
    c = 1.0 / S2
    for i in range(hn):
        Wm[i, 2 * i] = c
        Wm[i, 2 * i + 1] = c
        Wm[hn + i, 2 * i] = c
        Wm[hn + i, 2 * i + 1] = -c
    return Wm


def _momentum_coeffs():
    t = 1.0
    out = []
    for _ in range(MAX_ITER):
        t_new = (1.0 + math.sqrt(1.0 + 4.0 * t * t)) / 2.0
        out.append((t - 1.0) / t_new)
        t = t_new
    return out


def _pack_p6(x):
    """x: (2, 320, 320) -> (128, 6, 320), zero-padded dead region."""
    out = np.zeros((128, 6, 320), dtype=x.dtype)
    for ch in range(2):
        out[:, 3 * ch + 0] = x[ch, 0:128]
        out[:, 3 * ch + 1] = x[ch, 128:256]
        out[0:64, 3 * ch + 2] = x[ch, 256:320]
    return out


def _unpack_p6(p):
    out = np.zeros((2, 320, 320), dtype=p.dtype)
    for ch in range(2):
        out[ch, 0:128] = p[:, 3 * ch + 0]
        out[ch, 128:256] = p[:, 3 * ch + 1]
        out[ch, 256:320] = p[0:64, 3 * ch + 2]
    return out


def _host_consts():
    F, G = _dft_mats()
    Dd, Dg = _tv_mats()
    W1, W2, W3 = _haar_mat(320), _haar_mat(160), _haar_mat(80)
    f16 = np.float16
    return {
        "ftr": F.real.T.astype(f16), "fti": F.imag.T.astype(f16),
        "ftin": (-F.imag.T).astype(f16),
        "ifr": G.real.T.astype(f16), "ifi": G.imag.T.astype(f16),
        "ifin": (-G.imag.T).astype(f16),
        "ddt": Dd.T.astype(f16), "dgt": Dg.T.astype(f16),
        "w1t": W1.T.astype(f16), "w1h": (0.5 * W1).astype(f16),
        "w2t": W2.T.astype(f16), "w2h": (0.5 * W2).astype(f16),
        "w3t": W3.T.astype(f16), "w3h": (0.5 * W3).astype(f16),
    }


def _copy_segs(src_lay, dst_lay, nrows):
    out = {}
    for ch in (0, 1):
        def locate(lay, r):
            for (p0, p1, q, r0, r1) in lay[ch]:
                if r0 <= r < r1:
                    return p0 + (r - r0), q, r1 - r
            raise AssertionError(r)
        segs = []
        r = 0
        while r < nrows:
            sp, sq, sleft = locate(src_lay, r)
            dp, dq, dleft = locate(dst_lay, r)
            cnt = min(sleft, dleft, nrows - r)
            segs.append((sp, sq, dp, dq, cnt))
            r += cnt
        out[ch] = segs
    return out


SEG12 = _copy_segs(P6D, L2D, 160)
SEG23 = _copy_segs(L2D, L3D, 80)


def _build_nc():
    import concourse.bacc as bacc
    import concourse.tile as tile
    import concourse.mybir as mybir
    from contextlib import ExitStack

    dt = mybir.dt
    F32, F16 = dt.float32, dt.float16
    ALU = mybir.AluOpType
    AF = mybir.ActivationFunctionType

    lam = LAM_TV
    coeffs = _momentum_coeffs()
    lam_lvl = [LAM_WAV * (S2 ** (l + 1)) for l in range(LEVELS)]

    nc = bacc.Bacc("TRN2", target_bir_lowering=False, debug=False,
                   num_devices=NCORES)

    dr = {}
    for name in ("ftr", "fti", "ftin", "ifr", "ifi", "ifin", "w1t", "w1h",
                 "ddt", "dgt"):
        dr[name] = nc.dram_tensor(name, [320, 320], F16, kind="ExternalInput").ap()
    for name in ("w2t", "w2h"):
        dr[name] = nc.dram_tensor(name, [160, 160], F16, kind="ExternalInput").ap()
    for name in ("w3t", "w3h"):
        dr[name] = nc.dram_tensor(name, [80, 80], F16, kind="ExternalInput").ap()
    for i in range(IMGS):
        dr[f"y{i}"] = nc.dram_tensor(f"y{i}", [128, 6, 320], F16, kind="ExternalInput").ap()
        dr[f"ym{i}"] = nc.dram_tensor(f"ym{i}", [128, 6, 320], F32, kind="ExternalInput").ap()
        dr[f"mk{i}"] = nc.dram_tensor(f"mk{i}", [128, 6, 320], F16, kind="ExternalInput").ap()
        dr[f"xo{i}"] = nc.dram_tensor(f"xo{i}", [128, 6, 320], F32, kind="ExternalOutput").ap()

    with ExitStack() as ctx:
        tc = ctx.enter_context(tile.TileContext(nc))
        st = ctx.enter_context(tc.tile_pool(name="state", bufs=1))
        psa = ctx.enter_context(tc.tile_pool(name="psa", bufs=1, space="PSUM"))
        psb = ctx.enter_context(tc.tile_pool(name="psb", bufs=1, space="PSUM"))

        def T(tag, shape, dtp):
            return st.tile(shape, dtp, tag=tag, name=tag)

        cv = {}
        for name in ("ftr", "fti", "ftin", "ifr", "ifi", "ifin", "w1t", "w1h",
                     "ddt", "dgt"):
            cv[name] = T("c_" + name, [128, 3, 320], F16)
        for name in ("w2t", "w2h"):
            cv[name] = T("c_" + name, [128, 2, 160], F16)
        for name in ("w3t", "w3h"):
            cv[name] = T("c_" + name, [128, 2, 80], F16)

        def load_const(name, lay):
            for (p0, p1, q, r0, r1) in lay:
                nc.sync.dma_start(cv[name][p0:p1, q, :], dr[name][r0:r1, :])

        for name in ("ftr", "fti", "ftin", "ifr", "ifi", "ifin", "w1t", "w1h",
                     "ddt", "dgt"):
            load_const(name, P6C)
        for name in ("w2t", "w2h"):
            load_const(name, L2C)
        for name in ("w3t", "w3h"):
            load_const(name, L3C)

        per_img = []
        for i in range(IMGS):
            per_img.append({
                "z": T(f"z{i}", [128, 6, 320], F32),
                "zb": T(f"zb{i}", [128, 6, 320], F16),
                "xA": T(f"xA{i}", [128, 6, 320], F32),
                "xB": T(f"xB{i}", [128, 6, 320], F32),
                "ym": T(f"ymk{i}", [128, 6, 320], F32),
                "mk": T(f"msk{i}", [128, 6, 320], F16),
            })
        sbA = T("sbA", [128, 6, 320], F16)   # fft intermediate / xtv
        Kb = T("Kb", [128, 6, 320], F16)     # masked residual (ifft input)
        Km = T("Km", [128, 6, 320], F32)     # xc / inverse-DWT fp32 buffer
        xcb = T("xcb", [128, 6, 320], F16)
        wtmp = T("wtmp", [128, 6, 320], F16)
        Y1 = T("Y1", [128, 6, 320], F16)
        Y2 = T("Y2", [128, 4, 160], F16)
        Y3 = T("Y3", [128, 4, 80], F16)
        L2t = T("L2t", [128, 4, 160], F16)
        L3t = T("L3t", [128, 4, 80], F16)
        qx = T("qx", [128, 6, 322], F16)
        qy = T("qy", [128, 6, 320], F16)
        vt = T("vt", [128, 6, 320], F16)
        tv1 = T("tv1", [128, 6, 320], F16)
        n2 = T("n2", [128, 6, 320], F16)
        rr = T("rr", [128, 6, 320], F16)

        def psA():
            return psa.tile([128, 6, 512], F32, tag="A", name="psA")

        def psB():
            return psb.tile([128, 4, 256], F32, tag="B", name="psB")

        nc.vector.memset(qx[:], 0.0)

        # ----- pair views: valid region of a P6-layout tensor as 2 APs -----
        def pv(t, c0=0, c1=None, wd=None):
            c1 = c1 if c1 is not None else (wd if wd is not None else t.shape[-1])
            r = t.rearrange("p (g b) w -> p g b w", g=2)
            return [r[0:128, 0, 0:2, c0:c1], r[0:128, 1, 0:2, c0:c1],
                    r[0:64, :, 2, c0:c1]]

        def pvs(t, c0, c1, step):
            """strided-column pair views (for DWT w ops)"""
            r = t.rearrange("p (g b) w -> p g b w", g=2)
            return [r[0:128, 0, 0:2, c0:c1:step], r[0:128, 1, 0:2, c0:c1:step],
                    r[0:64, :, 2, c0:c1:step]]

        def ew(fn, *views):
            for i in range(3):
                fn(*[v[i] for v in views])

        STT = nc.vector.scalar_tensor_tensor
        TT = nc.vector.tensor_tensor

        def soft_full(t, lam_l, m_ap):
            """soft-threshold full tile in place: t -= clamp(t, +-lam_l)"""
            nc.vector.tensor_scalar(m_ap, t[:], lam_l, -lam_l,
                                    ALU.min, ALU.max)
            TT(t[:], t[:], m_ap, ALU.subtract)

        # ---------- matmul emitters ----------
        def mm_fft(ps, data, terms, ncols=320):
            for oc in (0, 1):
                for (mp0, mp1, mq, mr0, mr1) in P6D[oc]:
                    mml = []
                    for (dch, cname) in terms[oc]:
                        cvt = cv[cname]
                        for t in range(3):
                            dp0, dp1, dq, _, _ = P6D[dch][t]
                            cp0, cp1, cq, _, _ = P6C[t]
                            mml.append((data[dp0:dp1, dq, mr0:mr1],
                                        cvt[cp0:cp1, cq, 0:ncols]))
                    n = len(mml)
                    for idx, (l, r) in enumerate(mml):
                        nc.tensor.matmul(ps[mp0:mp1, mq, 0:ncols], l, r,
                                         start=(idx == 0), stop=(idx == n - 1))

        def mm_left(ps, cname, data, dlay, clay, ncols, pairs=None):
            for ch in (0, 1):
                cvt = cv[cname]
                dts = dlay[ch]
                for (oi, (mp0, mp1, mq, mr0, mr1)) in enumerate(dlay[ch]):
                    use = [t for t in range(len(dts))
                           if pairs is None or (oi, t) in pairs]
                    n = len(use)
                    for k, t in enumerate(use):
                        dp0, dp1, dq, _, _ = dts[t]
                        cp0, cp1, cq, _, _ = clay[t]
                        nc.tensor.matmul(
                            ps[mp0:mp1, mq, 0:ncols],
                            cvt[cp0:cp1, cq, mr0:mr1],
                            data[dp0:dp1, dq, 0:ncols],
                            start=(k == 0), stop=(k == n - 1))

        def fft2(data, consts, out_ps):
            cr, ci, cin = consts
            p1 = psA()
            mm_fft(p1, data, {0: [(0, cr), (1, cin)], 1: [(0, ci), (1, cr)]})
            ew(nc.scalar.copy, pv(sbA), pv(p1, c1=320))
            mm_fft(out_ps, sbA, {0: [(0, cr), (1, cin)], 1: [(0, ci), (1, cr)]})

        FWD = ("ftr", "fti", "ftin")
        INV = ("ifr", "ifi", "ifin")

        # ---------- init ----------
        for i in range(IMGS):
            im = per_img[i]
            nc.sync.dma_start(Kb[:], dr[f"y{i}"][:])
            nc.sync.dma_start(im["ym"][:], dr[f"ym{i}"][:])
            nc.sync.dma_start(im["mk"][:], dr[f"mk{i}"][:])
            pG = psA()
            fft2(Kb, INV, pG)
            ew(nc.scalar.copy, pv(im["xA"]), pv(pG, c1=320))
            ew(nc.scalar.copy, pv(im["zb"]), pv(pG, c1=320))
            ew(nc.vector.tensor_copy, pv(im["z"]), pv(pG, c1=320))

        # ---------- FISTA ----------
        for k in range(MAX_ITER):
            for i in range(IMGS):
                im = per_img[i]
                xold = im["xA"] if k % 2 == 0 else im["xB"]
                xnew = im["xB"] if k % 2 == 0 else im["xA"]

                pK = psA()
                fft2(im["zb"], FWD, pK)
                ew(lambda o, a, b: TT(o, a, b, ALU.mult),
                   pv(Kb), pv(pK, c1=320), pv(im["mk"]))
                ew(lambda o, a, b: TT(o, a, b, ALU.subtract),
                   pv(Kb), pv(Kb), pv(im["ym"]))
                pG = psA()
                fft2(Kb, INV, pG)
                xc = Km
                ew(lambda o, a, b: TT(o, a, b, ALU.subtract),
                   pv(xc), pv(im["z"]), pv(pG, c1=320))
                ew(nc.scalar.copy, pv(xcb), pv(xc))

                # ---------- TV prox (dual p in unscaled p-units) ----------
                for it in range(0 if SKIP_TV else TV_ITERS):
                    if it == 0:
                        pT = psA()
                        mm_left(pT, "dgt", xcb, P6D, P6C, 320, TV_PAIRS)
                        ew(lambda o, a, b: TT(o, a, b, ALU.subtract),
                           pv(qx, 2, 321), pv(xcb, 1, 320), pv(xcb, 0, 319))
                        ew(nc.scalar.square, pv(tv1), pv(qx, 2, 322))
                        ew(nc.scalar.square, pv(vt), pv(pT, c1=320))
                        ew(lambda o, a, b: TT(o, a, b, ALU.add),
                           pv(n2), pv(tv1), pv(vt))
                        nc.vector.tensor_scalar(n2[:], n2[:], 16.0, 1.0 / 16.0,
                                                ALU.max, ALU.mult)
                        nc.scalar.activation(rr[:], n2[:],
                                             AF.Abs_reciprocal_sqrt, scale=1.0)
                        ew(lambda o, a, b: STT(o, a, 0.25, b, ALU.mult, ALU.mult),
                           pv(qx, 2, 321), pv(qx, 2, 321), pv(rr, 0, 319))
                        ew(lambda o, a, b: STT(o, a, 0.25, b, ALU.mult, ALU.mult),
                           pv(qy), pv(pT, c1=320), pv(rr))
                    else:
                        ew(lambda o, a, b: STT(o, a, -lam, b, ALU.mult, ALU.add),
                           pv(tv1), pv(qx, 2, 322), pv(xcb))
                        ew(lambda o, a, b: STT(o, a, lam, b, ALU.mult, ALU.add),
                           pv(tv1), pv(qx, 1, 321), pv(tv1))
                        pT = psA()
                        mm_left(pT, "ddt", qy, P6D, P6C, 320, TV_PAIRS)
                        ew(lambda o, a, b: STT(o, a, -lam, b, ALU.mult, ALU.add),
                           pv(vt), pv(pT, c1=320), pv(tv1))
                        pT2 = psA()
                        mm_left(pT2, "dgt", vt, P6D, P6C, 320, TV_PAIRS)
                        ew(lambda o, a, b: STT(o, a, TAU, b, ALU.mult, ALU.add),
                           pv(qx, 2, 321), pv(vt, 1, 320), pv(qx, 2, 321))
                        ew(lambda o, a, b: STT(o, a, -TAU, b, ALU.mult, ALU.add),
                           pv(qx, 2, 321), pv(vt, 0, 319), pv(qx, 2, 321))
                        ew(lambda o, a, b: STT(o, a, TAU, b, ALU.mult, ALU.add),
                           pv(qy), pv(pT2, c1=320), pv(qy))
                        ew(nc.scalar.square, pv(tv1), pv(qx, 2, 322))
                        nc.scalar.square(vt[:], qy[:])
                        ew(lambda o, a, b: TT(o, a, b, ALU.add),
                           pv(n2), pv(tv1), pv(vt))
                        nc.vector.tensor_scalar_max(n2[:], n2[:], 1.0)
                        nc.scalar.activation(rr[:], n2[:],
                                             AF.Abs_reciprocal_sqrt, scale=1.0)
                        ew(lambda o, a, b: TT(o, a, b, ALU.mult),
                           pv(qx, 2, 321), pv(qx, 2, 321), pv(rr, 0, 319))
                        TT(qy[:], qy[:], rr[:], ALU.mult)
                xtv = sbA
                if SKIP_TV:
                    ew(nc.vector.tensor_copy, pv(xtv), pv(xc))
                else:
                    pT = psA()
                    mm_left(pT, "ddt", qy, P6D, P6C, 320, TV_PAIRS)
                    ew(lambda o, a, b: STT(o, a, -lam, b, ALU.mult, ALU.add),
                       pv(xtv), pv(qx, 2, 322), pv(xc))
                    ew(lambda o, a, b: STT(o, a, lam, b, ALU.mult, ALU.add),
                       pv(xtv), pv(qx, 1, 321), pv(xtv))
                    ew(lambda o, a, b: STT(o, a, -lam, b, ALU.mult, ALU.add),
                       pv(xtv), pv(pT, c1=320), pv(xtv))

                if SKIP_DWT:
                    ew(nc.vector.tensor_copy, pv(xnew), pv(xtv))
                else:
                    # ---------- DWT forward ----------
                    ew(lambda o, a, b: TT(o, a, b, ALU.add),
                       pv(wtmp, 0, 160), pvs(xtv, 0, 320, 2), pvs(xtv, 1, 320, 2))
                    ew(lambda o, a, b: TT(o, a, b, ALU.subtract),
                       pv(wtmp, 160, 320), pvs(xtv, 0, 320, 2), pvs(xtv, 1, 320, 2))
                    pY = psA()
                    mm_left(pY, "w1t", wtmp, P6D, P6C, 320, W1_PAIRS)
                    ew(nc.scalar.copy, pv(Y1), pv(pY, c1=320))
                    for ch in (0, 1):
                        for (sp, sq, dp, dq, cnt) in SEG12[ch]:
                            TT(L2t[dp:dp + cnt, dq, 0:80],
                               Y1[sp:sp + cnt, sq, 0:160:2],
                               Y1[sp:sp + cnt, sq, 1:160:2], ALU.add)
                            TT(L2t[dp:dp + cnt, dq, 80:160],
                               Y1[sp:sp + cnt, sq, 0:160:2],
                               Y1[sp:sp + cnt, sq, 1:160:2], ALU.subtract)
                    pY2 = psB()
                    mm_left(pY2, "w2t", L2t, L2D, L2C, 160)
                    nc.scalar.copy(Y2[0:128, 0:4:2, :], pY2[0:128, 0:4:2, 0:160])
                    nc.scalar.copy(Y2[0:32, 1:4:2, :], pY2[0:32, 1:4:2, 0:160])
                    for ch in (0, 1):
                        for (sp, sq, dp, dq, cnt) in SEG23[ch]:
                            TT(L3t[dp:dp + cnt, dq, 0:40],
                               Y2[sp:sp + cnt, sq, 0:80:2],
                               Y2[sp:sp + cnt, sq, 1:80:2], ALU.add)
                            TT(L3t[dp:dp + cnt, dq, 40:80],
                               Y2[sp:sp + cnt, sq, 0:80:2],
                               Y2[sp:sp + cnt, sq, 1:80:2], ALU.subtract)
                    pY3 = psB()
                    mm_left(pY3, "w3t", L3t, L3D, L3C, 80)
                    nc.scalar.copy(Y3[0:64, 0:4:2, :], pY3[0:64, 0:4:2, 0:80])
                    nc.scalar.copy(Y3[0:16, 1:4:2, :], pY3[0:16, 1:4:2, 0:80])
                    # thresholds: save ll3, soft-threshold everything, restore
                    nc.scalar.copy(L3t[0:40, 0:4:2, 0:40], Y3[0:40, 0:4:2, 0:40])
                    soft_full(Y3, lam_lvl[2], wtmp[0:128, 0:4, 0:80])
                    nc.scalar.copy(Y3[0:40, 0:4:2, 0:40], L3t[0:40, 0:4:2, 0:40])
                    soft_full(Y2, lam_lvl[1], wtmp[0:128, 0:4, 0:160])
                    soft_full(Y1, lam_lvl[0], wtmp[:])

                    # ---------- DWT inverse ----------
                    pZ3 = psB()
                    mm_left(pZ3, "w3h", Y3, L3D, L3C, 80)
                    nc.scalar.copy(L3t[0:64, 0:4:2, :], pZ3[0:64, 0:4:2, 0:80])
                    nc.scalar.copy(L3t[0:16, 1:4:2, :], pZ3[0:16, 1:4:2, 0:80])
                    for ch in (0, 1):
                        for (sp, sq, dp, dq, cnt) in SEG23[ch]:
                            TT(Y2[sp:sp + cnt, sq, 0:80:2],
                               L3t[dp:dp + cnt, dq, 0:40],
                               L3t[dp:dp + cnt, dq, 40:80], ALU.add)
                            TT(Y2[sp:sp + cnt, sq, 1:80:2],
                               L3t[dp:dp + cnt, dq, 0:40],
                               L3t[dp:dp + cnt, dq, 40:80], ALU.subtract)
                    pZ2 = psB()
                    mm_left(pZ2, "w2h", Y2, L2D, L2C, 160)
                    nc.scalar.copy(L2t[0:128, 0:4:2, :], pZ2[0:128, 0:4:2, 0:160])
                    nc.scalar.copy(L2t[0:32, 1:4:2, :], pZ2[0:32, 1:4:2, 0:160])
                    for ch in (0, 1):
                        for (sp, sq, dp, dq, cnt) in SEG12[ch]:
                            TT(Y1[sp:sp + cnt, sq, 0:160:2],
                               L2t[dp:dp + cnt, dq, 0:80],
                               L2t[dp:dp + cnt, dq, 80:160], ALU.add)
                            TT(Y1[sp:sp + cnt, sq, 1:160:2],
                               L2t[dp:dp + cnt, dq, 0:80],
                               L2t[dp:dp + cnt, dq, 80:160], ALU.subtract)
                    pZ1 = psA()
                    mm_left(pZ1, "w1h", Y1, P6D, P6C, 320, W1_PAIRS)
                    ew(nc.scalar.copy, pv(Km), pv(pZ1, c1=320))
                    ew(lambda o, a, b: TT(o, a, b, ALU.add),
                       pvs(xnew, 0, 320, 2), pv(Km, 0, 160), pv(Km, 160, 320))
                    ew(lambda o, a, b: TT(o, a, b, ALU.subtract),
                       pvs(xnew, 1, 320, 2), pv(Km, 0, 160), pv(Km, 160, 320))

                # ---------- momentum ----------
                if k < MAX_ITER - 1:
                    TT(wtmp[:], xnew[:], xold[:], ALU.subtract)
                    STT(im["z"][:], wtmp[:], coeffs[k], xnew[:],
                        ALU.mult, ALU.add)
                    nc.scalar.copy(im["zb"][:], im["z"][:])

        fin = "xB" if (MAX_ITER - 1) % 2 == 0 else "xA"
        for i in range(IMGS):
            nc.sync.dma_start(dr[f"xo{i}"][:], per_img[i][fin][:])

    nc.compile()
    return nc


_NC = None


def _get_nc():
    global _NC
    if _NC is None:
        _NC = _build_nc()
    return _NC


def _build_in_maps(y, mask):
    c = _host_consts()
    in_maps = []
    for core in range(NCORES):
        m = dict(c)
        for i in range(IMGS):
            b = core * IMGS + i
            mpair = np.broadcast_to(mask[b], (2, 320, 320)).astype(np.float16)
            m[f"y{i}"] = _pack_p6(y[b].astype(np.float16))
            m[f"ym{i}"] = _pack_p6((mask[b] * y[b]).astype(np.float32))
            m[f"mk{i}"] = _pack_p6(mpair)
        in_maps.append(m)
    return in_maps


def kernel(y, mask):
    from concourse.bass_utils import run_bass_kernel_spmd

    y = np.asarray(y, dtype=np.float32)
    mask = np.asarray(mask, dtype=np.float32)
    nc = _get_nc()
    in_maps = _build_in_maps(y, mask)

    res = run_bass_kernel_spmd(nc, in_maps, list(range(NCORES)))
    out = np.zeros((B, 2, H, W), dtype=np.float32)
    for core in range(NCORES):
        for i in range(IMGS):
            out[core * IMGS + i] = _unpack_p6(res.results[core][f"xo{i}"])
    return out


# revision 16
# speedup vs baseline: 1.7386x; 1.3420x over previous
"""Trainium2 Bass kernel for nn_CombinedCS (FISTA compressed-sensing recon).

Self-contained: hardcodes shapes (B=16, H=W=320), shards batch over 8 cores
(2 images per core), runs the full 15-iteration FISTA loop SBUF-resident.

v3 design:
  - centered 2D FFT as two PE matmul stages against the DFT matrix F
    (transpose-free: data is always lhsT, F^T always rhs), all operands fp16
  - incremental k-space residual: rf (fp32) += mask*F(dz) with dz = z_k+1 -
    z_k, so fp16 rounding error injected per iteration scales with |dz| and
    |rf| (both decay) instead of |z|; g = IF(rf)
  - TV prox dual stored as qt = tau*lam*p; rsqrt activation scale folds all
    unit conversions so the inner loop is TT ops (fp16 2x DVE mode) plus one
    STT; final iteration's rsqrt folds the 1/tau of the prox application
  - 3-level Haar DWT: w-step on DVE, h-step on PE; soft-threshold via
    x - clamp(x,+-t)
  - the two images are emitted stage-interleaved so PE work of one image
    overlaps DVE/ACT work of the other; PSUM pools are per-channel
    ([128,3,512] x2 bufs + [128,2,256] x2 bufs = 8 banks)

Layout P6: one complex image (2 ch x 320 x 320) packs into
[128 partitions, 6 blocks, 320]; channel ch occupies blocks 3ch..3ch+2
with h = 128*qb + p (qb block-in-channel; block 3ch+2 uses p<64 only).
"""
import math
import os

import numpy as np

H = W = 320
B = 16
NCORES = 8
IMGS = B // NCORES  # 2
LAM_TV = 0.005
LAM_WAV = 0.005
TAU = 0.25
TV_ITERS = 5
LEVELS = 3
MAX_ITER = int(os.environ.get("CS_ITERS", "15"))
SKIP_TV = os.environ.get("CS_SKIP_TV", "0") == "1"
SKIP_DWT = os.environ.get("CS_SKIP_DWT", "0") == "1"
S2 = math.sqrt(2.0)

P6D = {
    0: [(0, 128, 0, 0, 128), (0, 128, 1, 128, 256), (0, 64, 2, 256, 320)],
    1: [(0, 128, 3, 0, 128), (0, 128, 4, 128, 256), (0, 64, 5, 256, 320)],
}
P6C = [(0, 128, 0, 0, 128), (0, 128, 1, 128, 256), (0, 64, 2, 256, 320)]
L2D = {
    0: [(0, 128, 0, 0, 128), (0, 32, 1, 128, 160)],
    1: [(0, 128, 2, 0, 128), (0, 32, 3, 128, 160)],
}
L2C = [(0, 128, 0, 0, 128), (0, 32, 1, 128, 160)]
L3D = {
    0: [(0, 64, 0, 0, 64), (0, 16, 1, 64, 80)],
    1: [(0, 64, 2, 0, 64), (0, 16, 3, 64, 80)],
}
L3C = [(0, 64, 0, 0, 64), (0, 16, 1, 64, 80)]

TV_PAIRS = {(0, 0), (1, 0), (1, 1), (2, 1), (2, 2)}
W1_PAIRS = {(0, 0), (0, 1), (1, 0), (1, 1), (1, 2), (2, 1), (2, 2)}


def _dft_mats():
    I = np.eye(H, dtype=np.complex128)
    F = np.fft.fftshift(
        np.fft.fft(np.fft.ifftshift(I, axes=0), axis=0, norm="ortho"), axes=0
    )
    G = np.conj(F).T
    return F, G


def _tv_mats():
    Dd = np.zeros((H, H))
    Dd[0, 0] = 1.0
    for h in range(1, H - 1):
        Dd[h, h] = 1.0
        Dd[h, h - 1] = -1.0
    Dd[H - 1, H - 2] = -1.0
    Dg = np.zeros((H, H))
    for h in range(H - 1):
        Dg[h, h] = -1.0
        Dg[h, h + 1] = 1.0
    return Dd, Dg


def _haar_mat(n):
    Wm = np.zeros((n, n))
    hn = n // 2
    c = 1.0 / S2
    for i in range(hn):
        Wm[i, 2 * i] = c
        Wm[i, 2 * i + 1] = c
        Wm[hn + i, 2 * i] = c
        Wm[hn + i, 2 * i + 1] = -c
    return Wm


def _momentum_coeffs():
    t = 1.0
    out = []
    for _ in range(MAX_ITER):
        t_new = (1.0 + math.sqrt(1.0 + 4.0 * t * t)) / 2.0
        out.append((t - 1.0) / t_new)
        t = t_new
    return out


def _pack_p6(x):
    out = np.zeros((128, 6, 320), dtype=x.dtype)
    for ch in range(2):
        out[:, 3 * ch + 0] = x[ch, 0:128]
        out[:, 3 * ch + 1] = x[ch, 128:256]
        out[0:64, 3 * ch + 2] = x[ch, 256:320]
    return out


def _unpack_p6(p):
    out = np.zeros((2, 320, 320), dtype=p.dtype)
    for ch in range(2):
        out[ch, 0:128] = p[:, 3 * ch + 0]
        out[ch, 128:256] = p[:, 3 * ch + 1]
        out[ch, 256:320] = p[0:64, 3 * ch + 2]
    return out


def _host_consts():
    F, G = _dft_mats()
    Dd, Dg = _tv_mats()
    W1, W2, W3 = _haar_mat(320), _haar_mat(160), _haar_mat(80)
    f16 = np.float16
    return {
        "ftr": F.real.T.astype(f16), "fti": F.imag.T.astype(f16),
        "ftin": (-F.imag.T).astype(f16),
        "ifr": G.real.T.astype(f16), "ifi": G.imag.T.astype(f16),
        "ifin": (-G.imag.T).astype(f16),
        "ddt": Dd.T.astype(f16), "dgt": Dg.T.astype(f16),
        "dgs": (TAU * LAM_TV * Dg.T).astype(f16),
        "w1t": W1.T.astype(f16), "w1h": (0.5 * W1).astype(f16),
        "w2t": W2.T.astype(f16), "w2h": (0.5 * W2).astype(f16),
        "w3t": W3.T.astype(f16), "w3h": (0.5 * W3).astype(f16),
    }


def _copy_segs(src_lay, dst_lay, nrows):
    out = {}
    for ch in (0, 1):
        def locate(lay, r):
            for (p0, p1, q, r0, r1) in lay[ch]:
                if r0 <= r < r1:
                    return p0 + (r - r0), q, r1 - r
            raise AssertionError(r)
        segs = []
        r = 0
        while r < nrows:
            sp, sq, sleft = locate(src_lay, r)
            dp, dq, dleft = locate(dst_lay, r)
            cnt = min(sleft, dleft, nrows - r)
            segs.append((sp, sq, dp, dq, cnt))
            r += cnt
        out[ch] = segs
    return out


SEG12 = _copy_segs(P6D, L2D, 160)
SEG23 = _copy_segs(L2D, L3D, 80)


def _build_nc():
    import concourse.bacc as bacc
    import concourse.tile as tile
    import concourse.mybir as mybir
    from contextlib import ExitStack

    dt = mybir.dt
    F32, F16 = dt.float32, dt.float16
    ALU = mybir.AluOpType
    AF = mybir.ActivationFunctionType

    lam = LAM_TV
    coeffs = _momentum_coeffs()
    lam_lvl = [LAM_WAV * (S2 ** (l + 1)) for l in range(LEVELS)]

    nc = bacc.Bacc("TRN2", target_bir_lowering=False, debug=False,
                   num_devices=NCORES)

    dr = {}
    for name in ("ftr", "fti", "ftin", "ifr", "ifi", "ifin", "w1t", "w1h",
                 "ddt", "dgt", "dgs"):
        dr[name] = nc.dram_tensor(name, [320, 320], F16, kind="ExternalInput").ap()
    for name in ("w2t", "w2h"):
        dr[name] = nc.dram_tensor(name, [160, 160], F16, kind="ExternalInput").ap()
    for name in ("w3t", "w3h"):
        dr[name] = nc.dram_tensor(name, [80, 80], F16, kind="ExternalInput").ap()
    for i in range(IMGS):
        dr[f"y{i}"] = nc.dram_tensor(f"y{i}", [128, 6, 320], F16, kind="ExternalInput").ap()
        dr[f"yl{i}"] = nc.dram_tensor(f"yl{i}", [128, 6, 320], F16, kind="ExternalInput").ap()
        dr[f"ym{i}"] = nc.dram_tensor(f"ym{i}", [128, 6, 320], F32, kind="ExternalInput").ap()
        dr[f"mk{i}"] = nc.dram_tensor(f"mk{i}", [128, 6, 320], F16, kind="ExternalInput").ap()
        dr[f"xo{i}"] = nc.dram_tensor(f"xo{i}", [128, 6, 320], F32, kind="ExternalOutput").ap()

    with ExitStack() as ctx:
        tc = ctx.enter_context(tile.TileContext(nc))
        st = ctx.enter_context(tc.tile_pool(name="state", bufs=1))
        psa = ctx.enter_context(tc.tile_pool(name="psa", bufs=2, space="PSUM"))
        psb = ctx.enter_context(tc.tile_pool(name="psb", bufs=2, space="PSUM"))

        def T(tag, shape, dtp):
            return st.tile(shape, dtp, tag=tag, name=tag)

        cv = {}
        for name in ("ftr", "fti", "ftin", "ifr", "ifi", "ifin", "w1t", "w1h",
                     "ddt", "dgt", "dgs"):
            cv[name] = T("c_" + name, [128, 3, 320], F16)
        for name in ("w2t", "w2h"):
            cv[name] = T("c_" + name, [128, 2, 160], F16)
        for name in ("w3t", "w3h"):
            cv[name] = T("c_" + name, [128, 2, 80], F16)

        def load_const(name, lay):
            for (p0, p1, q, r0, r1) in lay:
                nc.sync.dma_start(cv[name][p0:p1, q, :], dr[name][r0:r1, :])

        for name in ("ftr", "fti", "ftin", "ifr", "ifi", "ifin", "w1t", "w1h",
                     "ddt", "dgt", "dgs"):
            load_const(name, P6C)
        for name in ("w2t", "w2h"):
            load_const(name, L2C)
        for name in ("w3t", "w3h"):
            load_const(name, L3C)

        im = []
        for i in range(IMGS):
            im.append({
                "z": T(f"z{i}", [128, 6, 320], F32),
                "dz": T(f"dz{i}", [128, 6, 320], F16),
                "rf": T(f"rf{i}", [128, 6, 320], F32),
                "xA": T(f"xA{i}", [128, 6, 320], F32),
                "xB": T(f"xB{i}", [128, 6, 320], F32),
                "ym": T(f"ymk{i}", [128, 6, 320], F32),
                "mk": T(f"msk{i}", [128, 6, 320], F16),
                "sbA": T(f"sbA{i}", [128, 6, 320], F16),
                "Kb": T(f"Kb{i}", [128, 6, 320], F16),
                "Km": T(f"Km{i}", [128, 6, 320], F32),
                "xct": T(f"xct{i}", [128, 6, 320], F16),
                "wtmp": T(f"wtmp{i}", [128, 6, 320], F16),
                "Y1": T(f"Y1{i}", [128, 6, 320], F16),
                "Y2": T(f"Y2{i}", [128, 4, 160], F16),
                "Y3": T(f"Y3{i}", [128, 4, 80], F16),
                "L2t": T(f"L2t{i}", [128, 4, 160], F16),
                "L3t": T(f"L3t{i}", [128, 4, 80], F16),
                "qx": T(f"qx{i}", [128, 6, 322], F16),
                "qy": T(f"qy{i}", [128, 6, 320], F16),
                "vt": T(f"vt{i}", [128, 6, 320], F16),
                "tv1": T(f"tv1{i}", [128, 6, 320], F16),
            })

        def psA():
            # one channel of a 320-row output: 3 blocks x bank
            return psa.tile([128, 3, 512], F32, tag="A", name="psA")

        def psB():
            # one channel of a 160-row output: 2 blocks packed in one bank
            return psb.tile([128, 2, 256], F32, tag="B", name="psB")

        for i in range(IMGS):
            nc.vector.memset(im[i]["qx"][:], 0.0)

        # ---- views ----
        def pv(t, c0=0, c1=None):
            c1 = c1 if c1 is not None else t.shape[-1]
            r = t.rearrange("p (g b) w -> p g b w", g=2)
            return [r[0:128, 0, 0:2, c0:c1], r[0:128, 1, 0:2, c0:c1],
                    r[0:64, :, 2, c0:c1]]

        def pvs(t, c0, c1, step):
            r = t.rearrange("p (g b) w -> p g b w", g=2)
            return [r[0:128, 0, 0:2, c0:c1:step], r[0:128, 1, 0:2, c0:c1:step],
                    r[0:64, :, 2, c0:c1:step]]

        def sv4(t, c0=0, c1=None):
            c1 = c1 if c1 is not None else t.shape[-1]
            r = t.rearrange("p (g b) w -> p g b w", g=2)
            return [r[0:128, 0, 0:2, c0:c1], r[0:64, 0, 2, c0:c1],
                    r[0:128, 1, 0:2, c0:c1], r[0:64, 1, 2, c0:c1]]

        def ppv(pp, c1=320):
            return [pp[0][0:128, 0:2, 0:c1], pp[0][0:64, 2, 0:c1],
                    pp[1][0:128, 0:2, 0:c1], pp[1][0:64, 2, 0:c1]]

        def ew(fn, *views):
            for vs in zip(*views):
                fn(*vs)

        STT = nc.vector.scalar_tensor_tensor
        TT = nc.vector.tensor_tensor

        def soft_full(t, lam_l, m_ap):
            nc.vector.tensor_scalar(m_ap, t[:], lam_l, -lam_l,
                                    ALU.min, ALU.max)
            TT(t[:], t[:], m_ap, ALU.subtract)

        # ---- matmul emitters (per-channel psum pair pp = [ch0, ch1]) ----
        def mm_fft(pp, datas, terms, accum=False):
            for oc in (0, 1):
                for ti, (mp0, mp1, mq, mr0, mr1) in enumerate(P6D[oc]):
                    mml = []
                    for data in datas:
                        for (dch, cname) in terms[oc]:
                            cvt = cv[cname]
                            for t in range(3):
                                dp0, dp1, dq, _, _ = P6D[dch][t]
                                cp0, cp1, cq, _, _ = P6C[t]
                                mml.append((data[dp0:dp1, dq, mr0:mr1],
                                            cvt[cp0:cp1, cq, 0:320]))
                    n = len(mml)
                    for idx, (l, r) in enumerate(mml):
                        nc.tensor.matmul(pp[oc][mp0:mp1, ti, 0:320], l, r,
                                         start=(idx == 0 and not accum),
                                         stop=(idx == n - 1))

        def mm_left(pp, cname, data, dlay, clay, ncols, pairs=None):
            for ch in (0, 1):
                cvt = cv[cname]
                dts = dlay[ch]
                for (oi, (mp0, mp1, mq, mr0, mr1)) in enumerate(dlay[ch]):
                    use = [t for t in range(len(dts))
                           if pairs is None or (oi, t) in pairs]
                    n = len(use)
                    for kk, t in enumerate(use):
                        dp0, dp1, dq, _, _ = dts[t]
                        cp0, cp1, cq, _, _ = clay[t]
                        nc.tensor.matmul(
                            pp[ch][mp0:mp1, oi, 0:ncols],
                            cvt[cp0:cp1, cq, mr0:mr1],
                            data[dp0:dp1, dq, 0:ncols],
                            start=(kk == 0), stop=(kk == n - 1))

        FWD = ("ftr", "fti", "ftin")
        INV = ("ifr", "ifi", "ifin")

        def fterms(consts):
            cr, ci, cin = consts
            return {0: [(0, cr), (1, cin)], 1: [(0, ci), (1, cr)]}

        # ================= init =================
        for i in range(IMGS):
            nc.sync.dma_start(im[i]["Kb"][:], dr[f"y{i}"][:])
            nc.sync.dma_start(im[i]["dz"][:], dr[f"yl{i}"][:])
            nc.sync.dma_start(im[i]["ym"][:], dr[f"ym{i}"][:])
            nc.sync.dma_start(im[i]["mk"][:], dr[f"mk{i}"][:])

        pG = {}
        for i in range(IMGS):
            p1 = (psA(), psA())
            mm_fft(p1, [im[i]["Kb"], im[i]["dz"]], fterms(INV))
            ew(nc.scalar.copy, sv4(im[i]["sbA"]), ppv(p1))
        for i in range(IMGS):
            pG[i] = (psA(), psA())
            mm_fft(pG[i], [im[i]["sbA"]], fterms(INV))
            ew(nc.scalar.copy, sv4(im[i]["xA"]), ppv(pG[i]))
            ew(nc.scalar.copy, sv4(im[i]["Kb"]), ppv(pG[i]))
            ew(nc.vector.tensor_copy, sv4(im[i]["z"]), ppv(pG[i]))
            # lo part of x0 for the split-precision initial forward FFT
            ew(lambda o, a, b: TT(o, a, b, ALU.subtract),
               sv4(im[i]["dz"]), ppv(pG[i]), sv4(im[i]["Kb"]))
        for i in range(IMGS):
            pK = (psA(), psA())
            mm_fft(pK, [im[i]["Kb"], im[i]["dz"]], fterms(FWD))
            ew(nc.scalar.copy, sv4(im[i]["sbA"]), ppv(pK))
        for i in range(IMGS):
            pK = (psA(), psA())
            mm_fft(pK, [im[i]["sbA"]], fterms(FWD))
            ew(lambda o, a, b: TT(o, a, b, ALU.mult),
               sv4(im[i]["wtmp"]), ppv(pK), sv4(im[i]["mk"]))
            ew(lambda o, a, b: TT(o, a, b, ALU.subtract),
               sv4(im[i]["rf"]), sv4(im[i]["wtmp"]), sv4(im[i]["ym"]))
            nc.scalar.copy(im[i]["Kb"][:], im[i]["rf"][:])

        # ================= FISTA =================
        def S(fn, *args):
            for i in range(IMGS):
                fn(i, *args)

        pK = {}

        def st_fwd1(i):
            p1 = (psA(), psA())
            mm_fft(p1, [im[i]["dz"]], fterms(FWD))
            ew(nc.scalar.copy, sv4(im[i]["sbA"]), ppv(p1))

        def st_fwd2(i):
            pk = (psA(), psA())
            mm_fft(pk, [im[i]["sbA"]], fterms(FWD))
            ew(lambda o, a, b: TT(o, a, b, ALU.mult),
               sv4(im[i]["wtmp"]), ppv(pk), sv4(im[i]["mk"]))
            ew(lambda o, a, b: TT(o, a, b, ALU.add),
               sv4(im[i]["rf"]), sv4(im[i]["rf"]), sv4(im[i]["wtmp"]))
            nc.scalar.copy(im[i]["Kb"][:], im[i]["rf"][:])

        def st_inv1(i):
            p1 = (psA(), psA())
            mm_fft(p1, [im[i]["Kb"]], fterms(INV))
            ew(nc.scalar.copy, sv4(im[i]["sbA"]), ppv(p1))

        def st_inv2(i):
            pg = (psA(), psA())
            mm_fft(pg, [im[i]["sbA"]], fterms(INV))
            # xc = z - g ; xct = tau*xc
            ew(lambda o, a, b: TT(o, a, b, ALU.subtract),
               sv4(im[i]["Km"]), sv4(im[i]["z"]), ppv(pg))
            nc.scalar.mul(im[i]["xct"][:], im[i]["Km"][:], TAU)

        # ---- TV (dual qt = tau*lam*p; qx guard cols 0,1 and 321) ----
        def st_tv0a(i):
            v = im[i]
            pt = (psA(), psA())
            mm_left(pt, "dgt", v["xct"], P6D, P6C, 320, TV_PAIRS)
            ew(lambda o, a, b: TT(o, a, b, ALU.subtract),
               pv(v["qx"], 2, 321), pv(v["xct"], 1, 320), pv(v["xct"], 0, 319))
            ew(nc.scalar.square, pv(v["tv1"]), pv(v["qx"], 2, 322))
            ew(nc.scalar.square, sv4(v["vt"]), ppv(pt))
            ew(lambda o, a, b: TT(o, a, b, ALU.add),
               pv(v["tv1"]), pv(v["tv1"]), pv(v["vt"]))
            nc.vector.tensor_scalar_max(v["tv1"][:], v["tv1"][:], 1.0)
            # rr = tau*lam / max(|u|,1)  (u = tau*grad); qt = u*rr
            nc.scalar.activation(v["vt"][:], v["tv1"][:],
                                 AF.Abs_reciprocal_sqrt,
                                 scale=1.0 / (TAU * TAU * lam * lam))
            ew(lambda o, a, b: TT(o, a, b, ALU.mult),
               pv(v["qx"], 2, 321), pv(v["qx"], 2, 321), pv(v["vt"], 0, 319))
            ew(lambda o, a, b: TT(o, a, b, ALU.mult),
               sv4(v["qy"]), ppv(pt), sv4(v["vt"]))

        def st_tvA(i, it):
            v = im[i]
            # vt(tau*v) = xct - div(qt) = xct - qt[w] + qt[w-1]
            ew(lambda o, a, b: TT(o, a, b, ALU.subtract),
               pv(v["tv1"]), pv(v["xct"]), pv(v["qx"], 2, 322))
            ew(lambda o, a, b: TT(o, a, b, ALU.add),
               pv(v["tv1"]), pv(v["tv1"]), pv(v["qx"], 1, 321))
            pt = (psA(), psA())
            mm_left(pt, "ddt", v["qy"], P6D, P6C, 320, TV_PAIRS)
            ew(lambda o, a, b: TT(o, a, b, ALU.subtract),
               sv4(v["vt"]), sv4(v["tv1"]), ppv(pt))

        def st_tvB(i, it):
            v = im[i]
            pt2 = (psA(), psA())
            mm_left(pt2, "dgs", v["vt"], P6D, P6C, 320, TV_PAIRS)
            # qt += lam * grad(vt)
            ew(lambda o, a, b: TT(o, a, b, ALU.subtract),
               pv(v["tv1"], 0, 319), pv(v["vt"], 1, 320), pv(v["vt"], 0, 319))
            ew(lambda o, a, b: STT(o, a, TAU * lam, b, ALU.mult, ALU.add),
               pv(v["qx"], 2, 321), pv(v["tv1"], 0, 319), pv(v["qx"], 2, 321))
            ew(lambda o, a, b: TT(o, a, b, ALU.add),
               sv4(v["qy"]), sv4(v["qy"]), ppv(pt2))
            # normalize: p = p / max(|p|,1);  p^2 = (qt/(tau*lam))^2
            ew(lambda o, a: nc.scalar.activation(o, a, AF.Square,
                                                 scale=1.0 / (TAU * lam)),
               pv(v["tv1"]), pv(v["qx"], 2, 322))
            nc.scalar.activation(v["vt"][:], v["qy"][:], AF.Square,
                                 scale=1.0 / (TAU * lam))
            ew(lambda o, a, b: TT(o, a, b, ALU.add),
               pv(v["tv1"]), pv(v["tv1"]), pv(v["vt"]))
            nc.vector.tensor_scalar_max(v["tv1"][:], v["tv1"][:], 1.0)
            last = it == TV_ITERS - 1
            # last iter: fold the 1/tau of x_tv = xc - (1/tau)*div(qt)
            nc.scalar.activation(v["vt"][:], v["tv1"][:],
                                 AF.Abs_reciprocal_sqrt,
                                 scale=(TAU * TAU if last else 1.0))
            ew(lambda o, a, b: TT(o, a, b, ALU.mult),
               pv(v["qx"], 2, 321), pv(v["qx"], 2, 321), pv(v["vt"], 0, 319))
            TT(v["qy"][:], v["qy"][:], v["vt"][:], ALU.mult)

        def st_tvfin(i):
            v = im[i]
            pt = (psA(), psA())
            mm_left(pt, "ddt", v["qy"], P6D, P6C, 320, TV_PAIRS)
            ew(lambda o, a, b: TT(o, a, b, ALU.subtract),
               pv(v["sbA"]), pv(v["Km"]), pv(v["qx"], 2, 322))
            ew(lambda o, a, b: TT(o, a, b, ALU.add),
               pv(v["sbA"]), pv(v["sbA"]), pv(v["qx"], 1, 321))
            ew(lambda o, a, b: TT(o, a, b, ALU.subtract),
               sv4(v["sbA"]), sv4(v["sbA"]), ppv(pt))

        # ---- DWT ----
        def st_dwtf1(i):
            v = im[i]
            xtv = v["sbA"]
            ew(lambda o, a, b: TT(o, a, b, ALU.add),
               pv(v["wtmp"], 0, 160), pvs(xtv, 0, 320, 2), pvs(xtv, 1, 320, 2))
            ew(lambda o, a, b: TT(o, a, b, ALU.subtract),
               pv(v["wtmp"], 160, 320), pvs(xtv, 0, 320, 2), pvs(xtv, 1, 320, 2))
            py = (psA(), psA())
            mm_left(py, "w1t", v["wtmp"], P6D, P6C, 320, W1_PAIRS)
            ew(nc.scalar.copy, sv4(v["Y1"]), ppv(py))

        def st_dwtf2(i):
            v = im[i]
            for ch in (0, 1):
                for (sp, sq, dp, dq, cnt) in SEG12[ch]:
                    TT(v["L2t"][dp:dp + cnt, dq, 0:80],
                       v["Y1"][sp:sp + cnt, sq, 0:160:2],
                       v["Y1"][sp:sp + cnt, sq, 1:160:2], ALU.add)
                    TT(v["L2t"][dp:dp + cnt, dq, 80:160],
                       v["Y1"][sp:sp + cnt, sq, 0:160:2],
                       v["Y1"][sp:sp + cnt, sq, 1:160:2], ALU.subtract)
            py2 = (psB(), psB())
            mm_left(py2, "w2t", v["L2t"], L2D, L2C, 160)
            for ch in (0, 1):
                nc.scalar.copy(v["Y2"][0:128, 2 * ch, :],
                               py2[ch][0:128, 0, 0:160])
                nc.scalar.copy(v["Y2"][0:32, 2 * ch + 1, :],
                               py2[ch][0:32, 1, 0:160])

        def st_dwtf3(i):
            v = im[i]
            for ch in (0, 1):
                for (sp, sq, dp, dq, cnt) in SEG23[ch]:
                    TT(v["L3t"][dp:dp + cnt, dq, 0:40],
                       v["Y2"][sp:sp + cnt, sq, 0:80:2],
                       v["Y2"][sp:sp + cnt, sq, 1:80:2], ALU.add)
                    TT(v["L3t"][dp:dp + cnt, dq, 40:80],
                       v["Y2"][sp:sp + cnt, sq, 0:80:2],
                       v["Y2"][sp:sp + cnt, sq, 1:80:2], ALU.subtract)
            py3 = (psB(), psB())
            mm_left(py3, "w3t", v["L3t"], L3D, L3C, 80)
            for ch in (0, 1):
                nc.scalar.copy(v["Y3"][0:64, 2 * ch, :], py3[ch][0:64, 0, 0:80])
                nc.scalar.copy(v["Y3"][0:16, 2 * ch + 1, :], py3[ch][0:16, 1, 0:80])

        def st_thresh(i):
            v = im[i]
            nc.scalar.copy(v["L3t"][0:40, 0:4:2, 0:40], v["Y3"][0:40, 0:4:2, 0:40])
            soft_full(v["Y3"], lam_lvl[2], v["wtmp"][0:128, 0:4, 0:80])
            nc.scalar.copy(v["Y3"][0:40, 0:4:2, 0:40], v["L3t"][0:40, 0:4:2, 0:40])
            soft_full(v["Y2"], lam_lvl[1], v["wtmp"][0:128, 0:4, 0:160])
            soft_full(v["Y1"], lam_lvl[0], v["wtmp"][:])

        def st_dwti3(i):
            v = im[i]
            pz3 = (psB(), psB())
            mm_left(pz3, "w3h", v["Y3"], L3D, L3C, 80)
            for ch in (0, 1):
                nc.scalar.copy(v["L3t"][0:64, 2 * ch, :], pz3[ch][0:64, 0, 0:80])
                nc.scalar.copy(v["L3t"][0:16, 2 * ch + 1, :], pz3[ch][0:16, 1, 0:80])
            for ch in (0, 1):
                for (sp, sq, dp, dq, cnt) in SEG23[ch]:
                    TT(v["Y2"][sp:sp + cnt, sq, 0:80:2],
                       v["L3t"][dp:dp + cnt, dq, 0:40],
                       v["L3t"][dp:dp + cnt, dq, 40:80], ALU.add)
                    TT(v["Y2"][sp:sp + cnt, sq, 1:80:2],
                       v["L3t"][dp:dp + cnt, dq, 0:40],
                       v["L3t"][dp:dp + cnt, dq, 40:80], ALU.subtract)

        def st_dwti2(i):
            v = im[i]
            pz2 = (psB(), psB())
            mm_left(pz2, "w2h", v["Y2"], L2D, L2C, 160)
            for ch in (0, 1):
                nc.scalar.copy(v["L2t"][0:128, 2 * ch, :], pz2[ch][0:128, 0, 0:160])
                nc.scalar.copy(v["L2t"][0:32, 2 * ch + 1, :], pz2[ch][0:32, 1, 0:160])
            for ch in (0, 1):
                for (sp, sq, dp, dq, cnt) in SEG12[ch]:
                    TT(v["Y1"][sp:sp + cnt, sq, 0:160:2],
                       v["L2t"][dp:dp + cnt, dq, 0:80],
                       v["L2t"][dp:dp + cnt, dq, 80:160], ALU.add)
                    TT(v["Y1"][sp:sp + cnt, sq, 1:160:2],
                       v["L2t"][dp:dp + cnt, dq, 0:80],
                       v["L2t"][dp:dp + cnt, dq, 80:160], ALU.subtract)

        def st_dwti1(i, xnew):
            v = im[i]
            pz1 = (psA(), psA())
            mm_left(pz1, "w1h", v["Y1"], P6D, P6C, 320, W1_PAIRS)
            ew(nc.scalar.copy, sv4(v["Km"]), ppv(pz1))
            ew(lambda o, a, b: TT(o, a, b, ALU.add),
               pvs(xnew[i], 0, 320, 2), pv(v["Km"], 0, 160), pv(v["Km"], 160, 320))
            ew(lambda o, a, b: TT(o, a, b, ALU.subtract),
               pvs(xnew[i], 1, 320, 2), pv(v["Km"], 0, 160), pv(v["Km"], 160, 320))

        def st_mom(i, xnew, xold, ck):
            v = im[i]
            TT(v["wtmp"][:], xnew[i][:], xold[i][:], ALU.subtract)
            TT(v["tv1"][:], xnew[i][:], v["z"][:], ALU.subtract)
            STT(v["dz"][:], v["wtmp"][:], ck, v["tv1"][:], ALU.mult, ALU.add)
            TT(v["z"][:], v["z"][:], v["dz"][:], ALU.add)

        for k in range(MAX_ITER):
            xold = [im[i]["xA"] if k % 2 == 0 else im[i]["xB"] for i in range(IMGS)]
            xnew = [im[i]["xB"] if k % 2 == 0 else im[i]["xA"] for i in range(IMGS)]
            if k > 0:
                # k=0: rf is already exact from init (no dz yet)
                S(st_fwd1)
                S(st_fwd2)
            S(st_inv1)
            S(st_inv2)
            if not SKIP_TV:
                S(st_tv0a)
                for it in range(1, TV_ITERS):
                    S(st_tvA, it)
                    S(st_tvB, it)
                S(st_tvfin)
            else:
                for i in range(IMGS):
                    nc.scalar.copy(im[i]["sbA"][:], im[i]["Km"][:])
            if not SKIP_DWT:
                S(st_dwtf1)
                S(st_dwtf2)
                S(st_dwtf3)
                S(st_thresh)
                S(st_dwti3)
                S(st_dwti2)
                S(st_dwti1, xnew)
            else:
                for i in range(IMGS):
                    ew(nc.vector.tensor_copy, pv(xnew[i]), pv(im[i]["sbA"]))
            if k < MAX_ITER - 1:
                S(st_mom, xnew, xold, coeffs[k])

        fin = "xB" if (MAX_ITER - 1) % 2 == 0 else "xA"
        for i in range(IMGS):
            nc.sync.dma_start(dr[f"xo{i}"][:], im[i][fin][:])

    nc.compile()
    return nc


_NC = None


def _get_nc():
    global _NC
    if _NC is None:
        _NC = _build_nc()
    return _NC


def _build_in_maps(y, mask):
    c = _host_consts()
    in_maps = []
    for core in range(NCORES):
        m = dict(c)
        for i in range(IMGS):
            b = core * IMGS + i
            mpair = np.broadcast_to(mask[b], (2, 320, 320)).astype(np.float16)
            yh = y[b].astype(np.float16)
            m[f"y{i}"] = _pack_p6(yh)
            m[f"yl{i}"] = _pack_p6((y[b] - yh.astype(np.float32)).astype(np.float16))
            m[f"ym{i}"] = _pack_p6((mask[b] * y[b]).astype(np.float32))
            m[f"mk{i}"] = _pack_p6(mpair)
        in_maps.append(m)
    return in_maps


def kernel(y, mask):
    from concourse.bass_utils import run_bass_kernel_spmd

    y = np.asarray(y, dtype=np.float32)
    mask = np.asarray(mask, dtype=np.float32)
    nc = _get_nc()
    in_maps = _build_in_maps(y, mask)

    res = run_bass_kernel_spmd(nc, in_maps, list(range(NCORES)))
    out = np.zeros((B, 2, H, W), dtype=np.float32)
    for core in range(NCORES):
        for i in range(IMGS):
            out[core * IMGS + i] = _unpack_p6(res.results[core][f"xo{i}"])
    return out
